# revision 9
# baseline (speedup 1.0000x reference)
"""CausalFieldAttentionV2 on 8 TRN2 NeuronCores.

Math (per reference): qkv projection (q unused) -> per-head k magnitude ->
deposit = v * |k| -> scatter-add into a G=512-bin field by token position ->
circular causal conv along the field (FFT in the reference) -> gather back at
each token's bin -> output projection.

Key transformations used here:
- The FFT circular conv with a fixed kernel == multiplication by a circulant
  matrix with ~11 significant taps.  Scatter + conv fuse into one banded
  matrix CS[g, n] = k[(g - bin(n)) % G] applied to the deposits; per
  128-token tile only a <=64-wide g-window of CS is nonzero, so the whole
  scatter+conv is a windowed PE matmul against a host-precomputed compacted
  block.
- The output projection commutes with the gather (row replication), so it is
  applied on the tiny [G, D] field (16x fewer FLOPs than token space), then
  the gather is a 0/1 windowed PE matmul back to token space.

Sharding: 8 cores = batch (4) x head-group (2 x 8 heads).  Each core computes
a partial [N, D] output (its 8 heads' contribution); the host unshard step
sums the two partials per sample (bout is folded into the head-group-0 core
via a K=1 ones x bout matmul).

Compute dtype: bf16 matmul operands, fp32 PSUM accumulation and fp32
elementwise/bias math (measured end-to-end rel err ~5e-3 vs fp32 reference).
"""

from contextlib import ExitStack

import numpy as np
import ml_dtypes

import concourse.bass as bass
import concourse.mybir as mybir
import concourse.tile as tile
from concourse import bacc
from concourse.bass_utils import run_bass_kernel_spmd

B, N, D, H, hd, G = 4, 8192, 1024, 16, 64, 512
HG = 8            # heads per group
F = HG * hd       # 512 features per head-group
T = 128           # tokens per tile
NT = N // T       # 64 token tiles
DC = D // T       # 8 contraction chunks for the kv matmul
EC = F // T       # 4 feature chunks
SIGMA = 0.5
CST_W = 64        # compacted scatter+conv block width
NCORES = 8

bf16 = ml_dtypes.bfloat16
f32 = np.float32


# ---------------------------------------------------------------- host prep

def _field_index():
    seq = np.arange(N, dtype=np.float32)
    idx = (seq / np.float32(max(N - 1, 1)) * np.float32(G - 1)).astype(np.int32)
    return np.clip(idx, 0, G - 1)


def _causal_kernel():
    i = np.arange(G, dtype=np.float32)
    center = G // 2
    with np.errstate(over="ignore"):
        k = np.where(i <= center, np.exp(-(center - i) / np.float32(SIGMA)), 0.0)
    k = k.astype(np.float32)
    return k / (k.sum() + np.float32(1e-8))


def _scatter_plan():
    """Per token-tile: compacted CS block [T, CST_W] and PSUM column segments.

    CS[g, n] = k[(g - bin(n)) % G].  Returns (cstw [NT, T, CST_W] bf16,
    segs[ti] = [(g_start, col_start, width), ...]).
    """
    fidx = _field_index()
    k = _causal_kernel()
    taps = np.nonzero(k > 1e-12)[0]
    tmin, tmax = int(taps.min()), int(taps.max())
    cstw = np.zeros((NT, T, CST_W), np.float32)
    segs = []
    for ti in range(NT):
        b = fidx[ti * T:(ti + 1) * T]
        lo = int(b.min()) + tmin
        width = int(b.max()) + tmax - lo + 1
        assert width <= CST_W
        for j in range(T):
            for t in taps:
                cstw[ti, j, b[j] + t - lo] += k[t]
        lo_m = lo % G
        if lo_m + width <= G:
            segs.append([(lo_m, 0, width)])
        else:
            w1 = G - lo_m
            segs.append([(lo_m, 0, w1), (0, w1, width - w1)])
    return cstw.astype(bf16), segs


def _gather_plan():
    """Per token-tile: 128-aligned K=128 gather blocks (one per touched
    g-chunk, so lhsT and rhs share base partition 0).

    Returns (g2tw [NB, T, T] bf16, blocks[ti] = [(g_chunk, block_idx), ...]).
    """
    fidx = _field_index()
    g2_blocks = []
    blocks = []
    for ti in range(NT):
        b = fidx[ti * T:(ti + 1) * T]
        entry = []
        for gc in sorted(set(int(x) // T for x in np.unique(b))):
            blk = np.zeros((T, T), np.float32)
            for t in range(T):
                g = int(b[t])
                if g // T == gc:
                    blk[g - gc * T, t] = 1.0
            entry.append((gc, len(g2_blocks)))
            g2_blocks.append(blk)
        blocks.append(entry)
    g2tw = np.stack(g2_blocks).astype(bf16)
    return g2tw, blocks


_SCATTER = None
_GATHER = None


def _plans():
    global _SCATTER, _GATHER
    if _SCATTER is None:
        _SCATTER = _scatter_plan()
        _GATHER = _gather_plan()
    return _SCATTER, _GATHER


def _host_inputs(x, Wqkv, bqkv, Wout, bout):
    """Build the 8 per-core input maps."""
    (cstw, _), (g2tw, _) = _plans()
    cstw_flat = np.ascontiguousarray(cstw)             # [NT, T, CST_W] bf16
    g2tw_flat = np.ascontiguousarray(g2tw)             # [NB, 64, T] bf16

    xr = []
    for b in range(B):
        # xr[ti, p, dc*T + t] = x[b, ti*T + t, dc*T + p]
        a = np.ascontiguousarray(x[b].T)               # [D, N]
        a = a.reshape(DC, T, NT, T).transpose(2, 1, 0, 3).reshape(NT, T, DC * T)
        xr.append(np.ascontiguousarray(a.astype(bf16)))

    per_hg = []
    for hg in range(2):
        rk = slice(D + hg * F, D + (hg + 1) * F)
        rv = slice(2 * D + hg * F, 2 * D + (hg + 1) * F)
        # wk[p, dc*F + f] = Wqkv[D + hg*F + f, dc*T + p]
        wk = np.ascontiguousarray(
            Wqkv[rk].T.reshape(DC, T, F).transpose(1, 0, 2).reshape(T, DC * F)
        ).astype(bf16)
        wv = np.ascontiguousarray(
            Wqkv[rv].T.reshape(DC, T, F).transpose(1, 0, 2).reshape(T, DC * F)
        ).astype(bf16)
        # wo[p, ec*D + d] = Wout[d, hg*F + ec*T + p]
        wo = np.ascontiguousarray(
            Wout[:, hg * F:(hg + 1) * F].T.reshape(EC, T, D)
            .transpose(1, 0, 2).reshape(T, EC * D)
        ).astype(bf16)
        bk = np.ascontiguousarray(
            np.broadcast_to(bqkv[rk][None, :], (T, F))
        ).astype(f32)
        bv = np.ascontiguousarray(
            np.broadcast_to(bqkv[rv][None, :], (T, F))
        ).astype(f32)
        bo = (bout if hg == 0 else np.zeros_like(bout)).reshape(1, D).astype(bf16)
        per_hg.append((wk, wv, wo, bk, bv, np.ascontiguousarray(bo)))

    in_maps = []
    for core in range(NCORES):
        b, hg = divmod(core, 2)
        wk, wv, wo, bk, bv, bo = per_hg[hg]
        in_maps.append({
            "xr": xr[b], "cstw": cstw_flat, "g2tw": g2tw_flat,
            "wk": wk, "wv": wv, "wo": wo, "bk": bk, "bv": bv, "bo": bo,
        })
    return in_maps


# ---------------------------------------------------------------- device

def build_nc():
    (_, segs), (g2tw, gblocks) = _plans()
    NB = g2tw.shape[0]
    dt = mybir.dt

    nc = bacc.Bacc("TRN2", target_bir_lowering=False, debug=False,
                   num_devices=NCORES)

    xr = nc.dram_tensor("xr", [NT, T, DC * T], dt.bfloat16,
                        kind="ExternalInput").ap()
    cstw = nc.dram_tensor("cstw", [NT, T, CST_W], dt.bfloat16,
                          kind="ExternalInput").ap()
    g2tw = nc.dram_tensor("g2tw", [NB, T, T], dt.bfloat16,
                          kind="ExternalInput").ap()
    wk = nc.dram_tensor("wk", [T, DC * F], dt.bfloat16, kind="ExternalInput").ap()
    wv = nc.dram_tensor("wv", [T, DC * F], dt.bfloat16, kind="ExternalInput").ap()
    wo = nc.dram_tensor("wo", [T, EC * D], dt.bfloat16, kind="ExternalInput").ap()
    bk = nc.dram_tensor("bk", [T, F], dt.float32, kind="ExternalInput").ap()
    bv = nc.dram_tensor("bv", [T, F], dt.float32, kind="ExternalInput").ap()
    bo = nc.dram_tensor("bo", [1, D], dt.bfloat16, kind="ExternalInput").ap()
    out = nc.dram_tensor("out", [N, D], dt.float32, kind="ExternalOutput").ap()

    Square = mybir.ActivationFunctionType.Square

    with tile.TileContext(nc) as tc, ExitStack() as ctx:
        const = ctx.enter_context(tc.tile_pool(name="const", bufs=1))

        wk_sb = const.tile([T, DC * F], dt.bfloat16, tag="wk")
        wv_sb = const.tile([T, DC * F], dt.bfloat16, tag="wv")
        wo_sb = const.tile([T, EC * D], dt.bfloat16, tag="wo")
        bk_sb = const.tile([T, F], dt.float32, tag="bk")
        bv_sb = const.tile([T, F], dt.float32, tag="bv")
        bo_sb = const.tile([1, D], dt.bfloat16, tag="bo")
        g2_sb = const.tile([T, NB * T], dt.bfloat16, tag="g2")
        ones_sb = const.tile([1, T], dt.bfloat16, tag="ones")
        zrhs_sb = const.tile([1, F], dt.bfloat16, tag="zrhs")

        nc.sync.dma_start(wk_sb[:], wk[:])
        nc.sync.dma_start(wv_sb[:], wv[:])
        nc.sync.dma_start(bk_sb[:], bk[:])
        nc.sync.dma_start(bv_sb[:], bv[:])
        nc.vector.memset(ones_sb[:], 1.0)
        nc.vector.memset(zrhs_sb[:], 0.0)

        def load_tail_consts():
            # issued mid-main-loop: only needed by the pc/gather phases
            nc.sync.dma_start(wo_sb[:], wo[:])
            nc.sync.dma_start(bo_sb[:], bo[:])
            # g2_sb[p, bi*T + t] = g2tw[bi, p, t]
            nc.sync.dma_start(
                g2_sb[:].rearrange("p (b t) -> p b t", b=NB),
                g2tw.rearrange("b p t -> p b t"))

        fct_ctx = ExitStack()
        fct_pool = fct_ctx.enter_context(
            tc.tile_pool(name="fct", bufs=1, space="PSUM"))
        fct = [fct_pool.tile([T, F], dt.float32, tag=f"fct{ec}", name=f"fct{ec}")
               for ec in range(EC)]
        # deterministically zero the accumulators (clears has_written too)
        for ec in range(EC):
            nc.tensor.matmul(fct[ec][:], ones_sb[0:1, :], zrhs_sb[0:1, :],
                             start=True, stop=False, skip_group_check=True)

        with tc.tile_pool(name="xp", bufs=3) as xp, \
             tc.tile_pool(name="cp", bufs=3) as cp, \
             tc.tile_pool(name="kvp", bufs=2, space="PSUM") as kvp, \
             tc.tile_pool(name="kbvb", bufs=2) as kbvb, \
             tc.tile_pool(name="depp", bufs=3) as depp, \
             tc.tile_pool(name="small", bufs=3) as small:
            for ti in range(NT):
                x_t = xp.tile([T, DC * T], dt.bfloat16, tag="x")
                nc.sync.dma_start(x_t[:], xr[ti])
                c_t = cp.tile([T, CST_W], dt.bfloat16, tag="c")
                nc.sync.dma_start(c_t[:], cstw[ti])

                k_ps = kvp.tile([T, F], dt.float32, tag="k")
                v_ps = kvp.tile([T, F], dt.float32, tag="v")
                for dc in range(DC):
                    nc.tensor.matmul(
                        k_ps[:], x_t[:, dc * T:(dc + 1) * T],
                        wk_sb[:, dc * F:(dc + 1) * F],
                        start=(dc == 0), stop=(dc == DC - 1))
                    nc.tensor.matmul(
                        v_ps[:], x_t[:, dc * T:(dc + 1) * T],
                        wv_sb[:, dc * F:(dc + 1) * F],
                        start=(dc == 0), stop=(dc == DC - 1))

                kb = kbvb.tile([T, F], dt.float32, tag="kb")
                nc.vector.tensor_add(kb[:], k_ps[:], bk_sb[:])
                vb = kbvb.tile([T, F], dt.float32, tag="vb")
                nc.vector.tensor_add(vb[:], v_ps[:], bv_sb[:])

                sq = kbvb.tile([T, F], dt.float32, tag="sq")
                nc.scalar.activation(sq[:], kb[:], Square)
                mag2 = small.tile([T, HG], dt.float32, tag="mag2")
                nc.vector.reduce_sum(
                    mag2[:], sq[:].rearrange("p (h e) -> p h e", h=HG),
                    axis=mybir.AxisListType.X)
                mag = small.tile([T, HG], dt.float32, tag="mag")
                nc.scalar.sqrt(mag[:], mag2[:])

                dep = depp.tile([T, F], dt.bfloat16, tag="dep")
                mag_b = bass.AP(mag.tensor, mag.offset,
                                [list(mag.ap[0]), [1, HG], [0, hd]])
                nc.vector.tensor_mul(
                    dep[:].rearrange("p (h e) -> p h e", h=HG),
                    vb[:].rearrange("p (h e) -> p h e", h=HG),
                    mag_b)

                if ti == 2:
                    load_tail_consts()

                last_tile = ti == NT - 1
                for ec in range(EC):
                    for si, (g0, c0, w) in enumerate(segs[ti]):
                        nc.tensor.matmul(
                            fct[ec][:, g0:g0 + w],
                            dep[:, ec * T:(ec + 1) * T],
                            c_t[:, c0:c0 + w],
                            start=False,
                            stop=last_tile and si == len(segs[ti]) - 1,
                            skip_group_check=True)

        # field -> sbuf (bf16) for the output projection
        fct_sb = const.tile([T, EC * F], dt.bfloat16, tag="fct_sb")
        for ec in range(EC):
            nc.scalar.copy(fct_sb[:, ec * F:(ec + 1) * F], fct[ec][:])
        fct_ctx.close()  # free the 4 field PSUM banks before the pc phase

        # output projection on the field: pc[g, d] (+ bout on hg0 cores)
        pc_sb = [const.tile([T, D], dt.bfloat16, tag=f"pc{gc}", name=f"pc_sb{gc}")
                 for gc in range(4)]
        with tc.tile_pool(name="pcp", bufs=1, space="PSUM") as pcp:
            for gc in range(4):
                for dcn in range(2):
                    p = pcp.tile([T, F], dt.float32, tag=f"pc{gc}{dcn}", name=f"pcp{gc}{dcn}")
                    for ec in range(EC):
                        nc.tensor.matmul(
                            p[:],
                            fct_sb[:, ec * F + gc * T:ec * F + (gc + 1) * T],
                            wo_sb[:, ec * D + dcn * F:ec * D + (dcn + 1) * F],
                            start=(ec == 0), stop=False)
                    nc.tensor.matmul(
                        p[:], ones_sb[0:1, :], bo_sb[0:1, dcn * F:(dcn + 1) * F],
                        start=False, stop=True)
                    nc.scalar.copy(pc_sb[gc][:, dcn * F:(dcn + 1) * F], p[:])

        # gather back to token space + write out
        with tc.tile_pool(name="gat", bufs=4, space="PSUM") as gat, \
             tc.tile_pool(name="ob", bufs=3) as obp:
            for ti in range(NT):
                ob = obp.tile([T, D], dt.float32, tag="ob")
                for dcn in range(2):
                    gt = gat.tile([T, F], dt.float32, tag="gt")
                    nblk = len(gblocks[ti])
                    for bi, (gc, bidx) in enumerate(gblocks[ti]):
                        nc.tensor.matmul(
                            gt[:],
                            g2_sb[:, bidx * T:(bidx + 1) * T],
                            pc_sb[gc][:, dcn * F:(dcn + 1) * F],
                            start=(bi == 0), stop=(bi == nblk - 1))
                    nc.scalar.copy(ob[:, dcn * F:(dcn + 1) * F], gt[:])
                nc.sync.dma_start(out[ti * T:(ti + 1) * T, :], ob[:])

    nc.compile()
    return nc


_NC = None


def _compiled():
    global _NC
    if _NC is None:
        _NC = build_nc()
    return _NC


def kernel(x, Wqkv, bqkv, Wout, bout):
    x = np.asarray(x, dtype=np.float32)
    Wqkv = np.asarray(Wqkv, dtype=np.float32)
    bqkv = np.asarray(bqkv, dtype=np.float32)
    Wout = np.asarray(Wout, dtype=np.float32)
    bout = np.asarray(bout, dtype=np.float32)

    nc = _compiled()
    in_maps = _host_inputs(x, Wqkv, bqkv, Wout, bout)
    res = run_bass_kernel_spmd(nc, in_maps, core_ids=list(range(NCORES)))

    out = np.empty((B, N, D), np.float32)
    for b in range(B):
        out[b] = res.results[2 * b]["out"] + res.results[2 * b + 1]["out"]
    return out


def run_traced(x, Wqkv, bqkv, Wout, bout, **trace_kwargs):
    """Like kernel() but with NTFF tracing; returns (out, BassKernelResults)."""
    import ntff_shim  # noqa: F401  # registers the axon NTFF hook

    nc = _compiled()
    in_maps = _host_inputs(
        np.asarray(x, np.float32), np.asarray(Wqkv, np.float32),
        np.asarray(bqkv, np.float32), np.asarray(Wout, np.float32),
        np.asarray(bout, np.float32))
    res = run_bass_kernel_spmd(nc, in_maps, core_ids=list(range(NCORES)),
                               trace=True, **trace_kwargs)
    out = np.empty((B, N, D), np.float32)
    for b in range(B):
        out[b] = res.results[2 * b]["out"] + res.results[2 * b + 1]["out"]
    return out, res


# revision 11
# speedup vs baseline: 1.0018x; 1.0018x over previous
"""CausalFieldAttentionV2 on 8 TRN2 NeuronCores.

Math (per reference): qkv projection (q unused) -> per-head k magnitude ->
deposit = v * |k| -> scatter-add into a G=512-bin field by token position ->
circular causal conv along the field (FFT in the reference) -> gather back at
each token's bin -> output projection.

Key transformations used here:
- The FFT circular conv with a fixed kernel == multiplication by a circulant
  matrix with ~11 significant taps.  Scatter + conv fuse into one banded
  matrix CS[g, n] = k[(g - bin(n)) % G] applied to the deposits; per
  128-token tile only a <=64-wide g-window of CS is nonzero, so the whole
  scatter+conv is a windowed PE matmul against a host-precomputed compacted
  block.
- The output projection commutes with the gather (row replication), so it is
  applied on the tiny [G, D] field (16x fewer FLOPs than token space), then
  the gather is a 0/1 windowed PE matmul back to token space.

Sharding: 8 cores = batch (4) x head-group (2 x 8 heads).  Each core computes
a partial [N, D] output (its 8 heads' contribution); the host unshard step
sums the two partials per sample (bout is folded into the head-group-0 core
via a K=1 ones x bout matmul).

Compute dtype: bf16 matmul operands, fp32 PSUM accumulation and fp32
elementwise/bias math (measured end-to-end rel err ~5e-3 vs fp32 reference).
"""

from contextlib import ExitStack

import numpy as np
import ml_dtypes

import concourse.bass as bass
import concourse.mybir as mybir
import concourse.tile as tile
from concourse import bacc
from concourse.bass_utils import run_bass_kernel_spmd

B, N, D, H, hd, G = 4, 8192, 1024, 16, 64, 512
HG = 8            # heads per group
F = HG * hd       # 512 features per head-group
T = 128           # tokens per tile
NT = N // T       # 64 token tiles
DC = D // T       # 8 contraction chunks for the kv matmul
EC = F // T       # 4 feature chunks
SIGMA = 0.5
CST_W = 64        # compacted scatter+conv block width
NCORES = 8

bf16 = ml_dtypes.bfloat16
f32 = np.float32


# ---------------------------------------------------------------- host prep

def _field_index():
    seq = np.arange(N, dtype=np.float32)
    idx = (seq / np.float32(max(N - 1, 1)) * np.float32(G - 1)).astype(np.int32)
    return np.clip(idx, 0, G - 1)


def _causal_kernel():
    i = np.arange(G, dtype=np.float32)
    center = G // 2
    with np.errstate(over="ignore"):
        k = np.where(i <= center, np.exp(-(center - i) / np.float32(SIGMA)), 0.0)
    k = k.astype(np.float32)
    return k / (k.sum() + np.float32(1e-8))


def _scatter_plan():
    """Per token-tile: compacted CS block [T, CST_W] and PSUM column segments.

    CS[g, n] = k[(g - bin(n)) % G].  Returns (cstw [NT, T, CST_W] bf16,
    segs[ti] = [(g_start, col_start, width), ...]).
    """
    fidx = _field_index()
    k = _causal_kernel()
    taps = np.nonzero(k > 1e-12)[0]
    tmin, tmax = int(taps.min()), int(taps.max())
    cstw = np.zeros((NT, T, CST_W), np.float32)
    segs = []
    for ti in range(NT):
        b = fidx[ti * T:(ti + 1) * T]
        lo = int(b.min()) + tmin
        width = int(b.max()) + tmax - lo + 1
        assert width <= CST_W
        for j in range(T):
            for t in taps:
                cstw[ti, j, b[j] + t - lo] += k[t]
        lo_m = lo % G
        if lo_m + width <= G:
            segs.append([(lo_m, 0, width)])
        else:
            w1 = G - lo_m
            segs.append([(lo_m, 0, w1), (0, w1, width - w1)])
    return cstw.astype(bf16), segs


def _gather_plan():
    """Per token-tile: 128-aligned K=128 gather blocks (one per touched
    g-chunk, so lhsT and rhs share base partition 0).

    Returns (g2tw [NB, T, T] bf16, blocks[ti] = [(g_chunk, block_idx), ...]).
    """
    fidx = _field_index()
    g2_blocks = []
    blocks = []
    for ti in range(NT):
        b = fidx[ti * T:(ti + 1) * T]
        entry = []
        for gc in sorted(set(int(x) // T for x in np.unique(b))):
            blk = np.zeros((T, T), np.float32)
            for t in range(T):
                g = int(b[t])
                if g // T == gc:
                    blk[g - gc * T, t] = 1.0
            entry.append((gc, len(g2_blocks)))
            g2_blocks.append(blk)
        blocks.append(entry)
    g2tw = np.stack(g2_blocks).astype(bf16)
    return g2tw, blocks


_SCATTER = None
_GATHER = None


def _plans():
    global _SCATTER, _GATHER
    if _SCATTER is None:
        _SCATTER = _scatter_plan()
        _GATHER = _gather_plan()
    return _SCATTER, _GATHER


def _host_inputs(x, Wqkv, bqkv, Wout, bout):
    """Build the 8 per-core input maps."""
    (cstw, _), (g2tw, _) = _plans()
    cstw_flat = np.ascontiguousarray(cstw)             # [NT, T, CST_W] bf16
    g2tw_flat = np.ascontiguousarray(g2tw)             # [NB, 64, T] bf16

    xr = []
    for b in range(B):
        # xr[ti, p, dc*T + t] = x[b, ti*T + t, dc*T + p]
        a = np.ascontiguousarray(x[b].T)               # [D, N]
        a = a.reshape(DC, T, NT, T).transpose(2, 1, 0, 3).reshape(NT, T, DC * T)
        xr.append(np.ascontiguousarray(a.astype(bf16)))

    per_hg = []
    for hg in range(2):
        rk = slice(D + hg * F, D + (hg + 1) * F)
        rv = slice(2 * D + hg * F, 2 * D + (hg + 1) * F)
        # wk[p, dc*F + f] = Wqkv[D + hg*F + f, dc*T + p]
        wk = np.ascontiguousarray(
            Wqkv[rk].T.reshape(DC, T, F).transpose(1, 0, 2).reshape(T, DC * F)
        ).astype(bf16)
        wv = np.ascontiguousarray(
            Wqkv[rv].T.reshape(DC, T, F).transpose(1, 0, 2).reshape(T, DC * F)
        ).astype(bf16)
        # wo[p, ec*D + d] = Wout[d, hg*F + ec*T + p]
        wo = np.ascontiguousarray(
            Wout[:, hg * F:(hg + 1) * F].T.reshape(EC, T, D)
            .transpose(1, 0, 2).reshape(T, EC * D)
        ).astype(bf16)
        bk = np.ascontiguousarray(
            np.broadcast_to(bqkv[rk][None, :], (T, F))
        ).astype(f32)
        bv = np.ascontiguousarray(
            np.broadcast_to(bqkv[rv][None, :], (T, F))
        ).astype(f32)
        bo = (bout if hg == 0 else np.zeros_like(bout)).reshape(1, D).astype(bf16)
        per_hg.append((wk, wv, wo, bk, bv, np.ascontiguousarray(bo)))

    in_maps = []
    for core in range(NCORES):
        b, hg = divmod(core, 2)
        wk, wv, wo, bk, bv, bo = per_hg[hg]
        in_maps.append({
            "xr": xr[b], "cstw": cstw_flat, "g2tw": g2tw_flat,
            "wk": wk, "wv": wv, "wo": wo, "bk": bk, "bv": bv, "bo": bo,
        })
    return in_maps


# ---------------------------------------------------------------- device

def build_nc():
    (_, segs), (g2tw_np, gblocks) = _plans()
    NB = g2tw_np.shape[0]
    dt = mybir.dt

    # Back-half burst schedule.  The conv kernel's +G/2 center shift means
    # fct g-chunk gc's columns stop receiving contributions at main-loop
    # tile ~{2: 18, 3: 34, 0: 50, 1: 64}[gc]; emit each chunk's output
    # projection + gathers as soon as its columns are final so the output
    # DMA overlaps the remaining main loop.  A gather tile goes in the burst
    # of its latest-ready g-chunk.
    GC_RANK = {2: 0, 3: 1, 0: 2, 1: 3}
    RANK_GC = {r: gc for gc, r in GC_RANK.items()}
    BURST_AFTER = {0: 20, 1: 36, 2: 52}          # rank -> main-loop tile
    burst_tiles = {r: [] for r in range(4)}
    for ti in range(NT):
        burst_tiles[max(GC_RANK[gc] for gc, _ in gblocks[ti])].append(ti)

    nc = bacc.Bacc("TRN2", target_bir_lowering=False, debug=False,
                   num_devices=NCORES)

    xr = nc.dram_tensor("xr", [NT, T, DC * T], dt.bfloat16,
                        kind="ExternalInput").ap()
    cstw = nc.dram_tensor("cstw", [NT, T, CST_W], dt.bfloat16,
                          kind="ExternalInput").ap()
    g2tw = nc.dram_tensor("g2tw", [NB, T, T], dt.bfloat16,
                          kind="ExternalInput").ap()
    wk = nc.dram_tensor("wk", [T, DC * F], dt.bfloat16, kind="ExternalInput").ap()
    wv = nc.dram_tensor("wv", [T, DC * F], dt.bfloat16, kind="ExternalInput").ap()
    wo = nc.dram_tensor("wo", [T, EC * D], dt.bfloat16, kind="ExternalInput").ap()
    bk = nc.dram_tensor("bk", [T, F], dt.float32, kind="ExternalInput").ap()
    bv = nc.dram_tensor("bv", [T, F], dt.float32, kind="ExternalInput").ap()
    bo = nc.dram_tensor("bo", [1, D], dt.bfloat16, kind="ExternalInput").ap()
    out = nc.dram_tensor("out", [N, D], dt.float32, kind="ExternalOutput").ap()

    Square = mybir.ActivationFunctionType.Square

    with tile.TileContext(nc) as tc, ExitStack() as ctx:
        const = ctx.enter_context(tc.tile_pool(name="const", bufs=1))

        wk_sb = const.tile([T, DC * F], dt.bfloat16, tag="wk")
        wv_sb = const.tile([T, DC * F], dt.bfloat16, tag="wv")
        wo_sb = const.tile([T, EC * D], dt.bfloat16, tag="wo")
        bk_sb = const.tile([T, F], dt.float32, tag="bk")
        bv_sb = const.tile([T, F], dt.float32, tag="bv")
        bo_sb = const.tile([1, D], dt.bfloat16, tag="bo")
        g2_sb = const.tile([T, NB * T], dt.bfloat16, tag="g2")
        ones_sb = const.tile([1, T], dt.bfloat16, tag="ones")
        zrhs_sb = const.tile([1, F], dt.bfloat16, tag="zrhs")
        fct_sb = const.tile([T, EC * F], dt.bfloat16, tag="fct_sb")
        pc_sb = [const.tile([T, D], dt.bfloat16, tag=f"pc{gc}", name=f"pc_sb{gc}")
                 for gc in range(4)]

        # main-loop constants on the SP HWDGE ring (ahead of the x stream)
        nc.sync.dma_start(wk_sb[:], wk[:])
        nc.sync.dma_start(wv_sb[:], wv[:])
        nc.sync.dma_start(bk_sb[:], bk[:])
        nc.sync.dma_start(bv_sb[:], bv[:])
        # tail-phase constants on the ACT HWDGE ring (separate FIFO)
        nc.scalar.dma_start(wo_sb[:], wo[:])
        nc.scalar.dma_start(bo_sb[:], bo[:])
        # g2_sb[p, bi*T + t] = g2tw[bi, p, t]
        nc.scalar.dma_start(
            g2_sb[:].rearrange("p (b t) -> p b t", b=NB),
            g2tw.rearrange("b p t -> p b t"))
        nc.vector.memset(ones_sb[:], 1.0)
        nc.vector.memset(zrhs_sb[:], 0.0)

        fct_ctx = ExitStack()
        fct_pool = fct_ctx.enter_context(
            tc.tile_pool(name="fct", bufs=1, space="PSUM"))
        fct = [fct_pool.tile([T, F], dt.float32, tag=f"fct{ec}", name=f"fct{ec}")
               for ec in range(EC)]
        # deterministically zero the accumulators (clears has_written too)
        for ec in range(EC):
            nc.tensor.matmul(fct[ec][:], ones_sb[0:1, :], zrhs_sb[0:1, :],
                             start=True, stop=False, skip_group_check=True)

        with tc.tile_pool(name="xp", bufs=3) as xp, \
             tc.tile_pool(name="cp", bufs=3) as cp, \
             tc.tile_pool(name="kvp", bufs=2, space="PSUM") as kvp, \
             tc.tile_pool(name="kbvb", bufs=2) as kbvb, \
             tc.tile_pool(name="depp", bufs=3) as depp, \
             tc.tile_pool(name="small", bufs=3) as small, \
             tc.tile_pool(name="ob", bufs=3) as obp:

            def emit_burst(rank):
                gc = RANK_GC[rank]
                # harvest the finalized fct columns for this g-chunk
                for ec in range(EC):
                    nc.scalar.copy(
                        fct_sb[:, ec * F + gc * T:ec * F + (gc + 1) * T],
                        fct[ec][:, gc * T:(gc + 1) * T])
                # output projection on the field (+ bout fold, hg0 data only)
                for dcn in range(2):
                    p = kvp.tile([T, F], dt.float32, tag="k",
                                 name=f"pcp{gc}{dcn}")
                    for ec in range(EC):
                        nc.tensor.matmul(
                            p[:],
                            fct_sb[:, ec * F + gc * T:ec * F + (gc + 1) * T],
                            wo_sb[:, ec * D + dcn * F:ec * D + (dcn + 1) * F],
                            start=(ec == 0), stop=False)
                    nc.tensor.matmul(
                        p[:], ones_sb[0:1, :], bo_sb[0:1, dcn * F:(dcn + 1) * F],
                        start=False, stop=True)
                    nc.scalar.copy(pc_sb[gc][:, dcn * F:(dcn + 1) * F], p[:])
                # gather + write every token tile whose bins are now final
                for gti in burst_tiles[rank]:
                    ob = obp.tile([T, D], dt.float32, tag="ob", name=f"ob{gti}")
                    for dcn in range(2):
                        gt = kvp.tile([T, F], dt.float32, tag="v",
                                      name=f"gt{gti}{dcn}")
                        nblk = len(gblocks[gti])
                        for bi, (bgc, bidx) in enumerate(gblocks[gti]):
                            nc.tensor.matmul(
                                gt[:],
                                g2_sb[:, bidx * T:(bidx + 1) * T],
                                pc_sb[bgc][:, dcn * F:(dcn + 1) * F],
                                start=(bi == 0), stop=(bi == nblk - 1))
                        nc.scalar.copy(ob[:, dcn * F:(dcn + 1) * F], gt[:])
                    nc.sync.dma_start(out[gti * T:(gti + 1) * T, :], ob[:])

            for ti in range(NT):
                x_t = xp.tile([T, DC * T], dt.bfloat16, tag="x")
                nc.sync.dma_start(x_t[:], xr[ti])
                c_t = cp.tile([T, CST_W], dt.bfloat16, tag="c")
                nc.sync.dma_start(c_t[:], cstw[ti])

                k_ps = kvp.tile([T, F], dt.float32, tag="k")
                v_ps = kvp.tile([T, F], dt.float32, tag="v")
                for dc in range(DC):
                    nc.tensor.matmul(
                        k_ps[:], x_t[:, dc * T:(dc + 1) * T],
                        wk_sb[:, dc * F:(dc + 1) * F],
                        start=(dc == 0), stop=(dc == DC - 1))
                    nc.tensor.matmul(
                        v_ps[:], x_t[:, dc * T:(dc + 1) * T],
                        wv_sb[:, dc * F:(dc + 1) * F],
                        start=(dc == 0), stop=(dc == DC - 1))

                kb = kbvb.tile([T, F], dt.float32, tag="kb")
                nc.vector.tensor_add(kb[:], k_ps[:], bk_sb[:])
                vb = kbvb.tile([T, F], dt.float32, tag="vb")
                nc.vector.tensor_add(vb[:], v_ps[:], bv_sb[:])

                sq = kbvb.tile([T, F], dt.float32, tag="sq")
                nc.scalar.activation(sq[:], kb[:], Square)
                mag2 = small.tile([T, HG], dt.float32, tag="mag2")
                nc.vector.reduce_sum(
                    mag2[:], sq[:].rearrange("p (h e) -> p h e", h=HG),
                    axis=mybir.AxisListType.X)
                mag = small.tile([T, HG], dt.float32, tag="mag")
                nc.scalar.sqrt(mag[:], mag2[:])

                dep = depp.tile([T, F], dt.bfloat16, tag="dep")
                mag_b = bass.AP(mag.tensor, mag.offset,
                                [list(mag.ap[0]), [1, HG], [0, hd]])
                nc.gpsimd.tensor_mul(
                    dep[:].rearrange("p (h e) -> p h e", h=HG),
                    vb[:].rearrange("p (h e) -> p h e", h=HG),
                    mag_b)

                last_tile = ti == NT - 1
                for ec in range(EC):
                    for si, (g0, c0, w) in enumerate(segs[ti]):
                        nc.tensor.matmul(
                            fct[ec][:, g0:g0 + w],
                            dep[:, ec * T:(ec + 1) * T],
                            c_t[:, c0:c0 + w],
                            start=False,
                            stop=last_tile and si == len(segs[ti]) - 1,
                            skip_group_check=True)

                for rank, after in BURST_AFTER.items():
                    if ti == after:
                        emit_burst(rank)

            emit_burst(3)   # gc1: needs the final tokens
        fct_ctx.close()

    nc.compile()
    return nc


_NC = None


def _compiled():
    global _NC
    if _NC is None:
        _NC = build_nc()
    return _NC


def kernel(x, Wqkv, bqkv, Wout, bout):
    x = np.asarray(x, dtype=np.float32)
    Wqkv = np.asarray(Wqkv, dtype=np.float32)
    bqkv = np.asarray(bqkv, dtype=np.float32)
    Wout = np.asarray(Wout, dtype=np.float32)
    bout = np.asarray(bout, dtype=np.float32)

    nc = _compiled()
    in_maps = _host_inputs(x, Wqkv, bqkv, Wout, bout)
    res = run_bass_kernel_spmd(nc, in_maps, core_ids=list(range(NCORES)))

    out = np.empty((B, N, D), np.float32)
    for b in range(B):
        out[b] = res.results[2 * b]["out"] + res.results[2 * b + 1]["out"]
    return out


def run_traced(x, Wqkv, bqkv, Wout, bout, **trace_kwargs):
    """Like kernel() but with NTFF tracing; returns (out, BassKernelResults)."""
    import ntff_shim  # noqa: F401  # registers the axon NTFF hook

    nc = _compiled()
    in_maps = _host_inputs(
        np.asarray(x, np.float32), np.asarray(Wqkv, np.float32),
        np.asarray(bqkv, np.float32), np.asarray(Wout, np.float32),
        np.asarray(bout, np.float32))
    res = run_bass_kernel_spmd(nc, in_maps, core_ids=list(range(NCORES)),
                               trace=True, **trace_kwargs)
    out = np.empty((B, N, D), np.float32)
    for b in range(B):
        out[b] = res.results[2 * b]["out"] + res.results[2 * b + 1]["out"]
    return out, res


# revision 12
# speedup vs baseline: 1.2748x; 1.2724x over previous
"""CausalFieldAttentionV2 on 8 TRN2 NeuronCores.

Math (per reference): qkv projection (q unused) -> per-head k magnitude ->
deposit = v * |k| -> scatter-add into a G=512-bin field by token position ->
circular causal conv along the field (FFT in the reference) -> gather back at
each token's bin -> output projection.

Key transformations used here:
- The FFT circular conv with a fixed kernel == multiplication by a circulant
  matrix with ~11 significant taps.  Scatter + conv fuse into one banded
  matrix CS[g, n] = k[(g - bin(n)) % G] applied to the deposits; per
  128-token tile only a <=64-wide g-window of CS is nonzero, so the whole
  scatter+conv is a windowed PE matmul against a host-precomputed compacted
  block.
- The output projection commutes with the gather (row replication), so it is
  applied on the tiny [G, D] field (16x fewer FLOPs than token space), then
  the gather is a 0/1 windowed PE matmul back to token space.

Sharding: 8 cores = batch (4) x head-group (2 x 8 heads).  Each core computes
a partial [N, D] output (its 8 heads' contribution); the host unshard step
sums the two partials per sample (bout is folded into the head-group-0 core
via a K=1 ones x bout matmul).

Compute dtype: bf16 matmul operands, fp32 PSUM accumulation and fp32
elementwise/bias math (measured end-to-end rel err ~5e-3 vs fp32 reference).
"""

from contextlib import ExitStack

import numpy as np
import ml_dtypes

import concourse.bass as bass
import concourse.mybir as mybir
import concourse.tile as tile
from concourse import bacc
from concourse.bass_utils import run_bass_kernel_spmd

B, N, D, H, hd, G = 4, 8192, 1024, 16, 64, 512
HG = 8            # heads per group
F = HG * hd       # 512 features per head-group
T = 128           # tokens per tile
NT = N // T       # 64 token tiles
DC = D // T       # 8 contraction chunks for the kv matmul
EC = F // T       # 4 feature chunks
SIGMA = 0.5
CST_W = 64        # compacted scatter+conv block width
NCORES = 8

bf16 = ml_dtypes.bfloat16
f32 = np.float32


# ---------------------------------------------------------------- host prep

def _field_index():
    seq = np.arange(N, dtype=np.float32)
    idx = (seq / np.float32(max(N - 1, 1)) * np.float32(G - 1)).astype(np.int32)
    return np.clip(idx, 0, G - 1)


def _causal_kernel():
    i = np.arange(G, dtype=np.float32)
    center = G // 2
    with np.errstate(over="ignore"):
        k = np.where(i <= center, np.exp(-(center - i) / np.float32(SIGMA)), 0.0)
    k = k.astype(np.float32)
    return k / (k.sum() + np.float32(1e-8))


def _scatter_plan():
    """Per token-tile: compacted CS block [T, CST_W] and PSUM column segments.

    CS[g, n] = k[(g - bin(n)) % G].  Returns (cstw [NT, T, CST_W] bf16,
    segs[ti] = [(g_start, col_start, width), ...]).
    """
    fidx = _field_index()
    k = _causal_kernel()
    taps = np.nonzero(k > 1e-12)[0]
    tmin, tmax = int(taps.min()), int(taps.max())
    cstw = np.zeros((NT, T, CST_W), np.float32)
    segs = []
    for ti in range(NT):
        b = fidx[ti * T:(ti + 1) * T]
        lo = int(b.min()) + tmin
        width = int(b.max()) + tmax - lo + 1
        assert width <= CST_W
        for j in range(T):
            for t in taps:
                cstw[ti, j, b[j] + t - lo] += k[t]
        lo_m = lo % G
        if lo_m + width <= G:
            segs.append([(lo_m, 0, width)])
        else:
            w1 = G - lo_m
            segs.append([(lo_m, 0, w1), (0, w1, width - w1)])
    return cstw.astype(bf16), segs


def _gather_plan():
    """Per token-tile: 128-aligned K=128 gather blocks (one per touched
    g-chunk, so lhsT and rhs share base partition 0).

    Returns (g2tw [NB, T, T] bf16, blocks[ti] = [(g_chunk, block_idx), ...]).
    """
    fidx = _field_index()
    g2_blocks = []
    blocks = []
    for ti in range(NT):
        b = fidx[ti * T:(ti + 1) * T]
        entry = []
        for gc in sorted(set(int(x) // T for x in np.unique(b))):
            blk = np.zeros((T, T), np.float32)
            for t in range(T):
                g = int(b[t])
                if g // T == gc:
                    blk[g - gc * T, t] = 1.0
            entry.append((gc, len(g2_blocks)))
            g2_blocks.append(blk)
        blocks.append(entry)
    g2tw = np.stack(g2_blocks).astype(bf16)
    return g2tw, blocks


_SCATTER = None
_GATHER = None


def _plans():
    global _SCATTER, _GATHER
    if _SCATTER is None:
        _SCATTER = _scatter_plan()
        _GATHER = _gather_plan()
    return _SCATTER, _GATHER


def _host_inputs(x, Wqkv, bqkv, Wout, bout):
    """Build the 8 per-core input maps."""
    (cstw, _), (g2tw, _) = _plans()
    cstw_flat = np.ascontiguousarray(cstw)             # [NT, T, CST_W] bf16
    g2tw_flat = np.ascontiguousarray(g2tw)             # [NB, 64, T] bf16

    xr = []
    for b in range(B):
        # xr[ti, p, dc*T + t] = x[b, ti*T + t, dc*T + p]
        a = np.ascontiguousarray(x[b].T)               # [D, N]
        a = a.reshape(DC, T, NT, T).transpose(2, 1, 0, 3).reshape(NT, T, DC * T)
        xr.append(np.ascontiguousarray(a.astype(bf16)))

    per_hg = []
    for hg in range(2):
        rk = slice(D + hg * F, D + (hg + 1) * F)
        rv = slice(2 * D + hg * F, 2 * D + (hg + 1) * F)
        # wk[p, dc*F + f] = Wqkv[D + hg*F + f, dc*T + p]
        wk = np.ascontiguousarray(
            Wqkv[rk].T.reshape(DC, T, F).transpose(1, 0, 2).reshape(T, DC * F)
        ).astype(bf16)
        wv = np.ascontiguousarray(
            Wqkv[rv].T.reshape(DC, T, F).transpose(1, 0, 2).reshape(T, DC * F)
        ).astype(bf16)
        # wo[p, ec*D + d] = Wout[d, hg*F + ec*T + p]
        wo = np.ascontiguousarray(
            Wout[:, hg * F:(hg + 1) * F].T.reshape(EC, T, D)
            .transpose(1, 0, 2).reshape(T, EC * D)
        ).astype(bf16)
        bk = np.ascontiguousarray(
            np.broadcast_to(bqkv[rk][None, :], (T, F))
        ).astype(f32)
        bv = np.ascontiguousarray(
            np.broadcast_to(bqkv[rv][None, :], (T, F))
        ).astype(f32)
        bo = (bout if hg == 0 else np.zeros_like(bout)).reshape(1, D).astype(bf16)
        per_hg.append((wk, wv, wo, bk, bv, np.ascontiguousarray(bo)))

    in_maps = []
    for core in range(NCORES):
        b, hg = divmod(core, 2)
        wk, wv, wo, bk, bv, bo = per_hg[hg]
        in_maps.append({
            "xr": xr[b], "cstw": cstw_flat, "g2tw": g2tw_flat,
            "wk": wk, "wv": wv, "wo": wo, "bk": bk, "bv": bv, "bo": bo,
        })
    return in_maps


# ---------------------------------------------------------------- device

def build_nc():
    (_, segs), (g2tw_np, gblocks) = _plans()
    NB = g2tw_np.shape[0]
    dt = mybir.dt

    # Back-half burst schedule.  The conv kernel's +G/2 center shift means
    # fct g-chunk gc's columns stop receiving contributions at main-loop
    # tile ~{2: 18, 3: 34, 0: 50, 1: 64}[gc]; emit each chunk's output
    # projection + gathers as soon as its columns are final so the output
    # DMA overlaps the remaining main loop.  A gather tile goes in the burst
    # of its latest-ready g-chunk.
    GC_RANK = {2: 0, 3: 1, 0: 2, 1: 3}
    RANK_GC = {r: gc for gc, r in GC_RANK.items()}
    BURST_AFTER = {0: 20, 1: 36, 2: 52}          # rank -> main-loop tile
    burst_tiles = {r: [] for r in range(4)}
    for ti in range(NT):
        burst_tiles[max(GC_RANK[gc] for gc, _ in gblocks[ti])].append(ti)

    nc = bacc.Bacc("TRN2", target_bir_lowering=False, debug=False,
                   num_devices=NCORES)

    xr = nc.dram_tensor("xr", [NT, T, DC * T], dt.bfloat16,
                        kind="ExternalInput").ap()
    cstw = nc.dram_tensor("cstw", [NT, T, CST_W], dt.bfloat16,
                          kind="ExternalInput").ap()
    g2tw = nc.dram_tensor("g2tw", [NB, T, T], dt.bfloat16,
                          kind="ExternalInput").ap()
    wk = nc.dram_tensor("wk", [T, DC * F], dt.bfloat16, kind="ExternalInput").ap()
    wv = nc.dram_tensor("wv", [T, DC * F], dt.bfloat16, kind="ExternalInput").ap()
    wo = nc.dram_tensor("wo", [T, EC * D], dt.bfloat16, kind="ExternalInput").ap()
    bk = nc.dram_tensor("bk", [T, F], dt.float32, kind="ExternalInput").ap()
    bv = nc.dram_tensor("bv", [T, F], dt.float32, kind="ExternalInput").ap()
    bo = nc.dram_tensor("bo", [1, D], dt.bfloat16, kind="ExternalInput").ap()
    out = nc.dram_tensor("out", [N, D], dt.bfloat16, kind="ExternalOutput").ap()

    Square = mybir.ActivationFunctionType.Square

    with tile.TileContext(nc) as tc, ExitStack() as ctx:
        const = ctx.enter_context(tc.tile_pool(name="const", bufs=1))

        wk_sb = const.tile([T, DC * F], dt.bfloat16, tag="wk")
        wv_sb = const.tile([T, DC * F], dt.bfloat16, tag="wv")
        wo_sb = const.tile([T, EC * D], dt.bfloat16, tag="wo")
        bk_sb = const.tile([T, F], dt.float32, tag="bk")
        bv_sb = const.tile([T, F], dt.float32, tag="bv")
        bo_sb = const.tile([1, D], dt.bfloat16, tag="bo")
        g2_sb = const.tile([T, NB * T], dt.bfloat16, tag="g2")
        ones_sb = const.tile([1, T], dt.bfloat16, tag="ones")
        zrhs_sb = const.tile([1, F], dt.bfloat16, tag="zrhs")
        fct_sb = const.tile([T, EC * F], dt.bfloat16, tag="fct_sb")
        pc_sb = [const.tile([T, D], dt.bfloat16, tag=f"pc{gc}", name=f"pc_sb{gc}")
                 for gc in range(4)]

        # main-loop constants on the SP HWDGE ring (ahead of the x stream)
        nc.sync.dma_start(wk_sb[:], wk[:])
        nc.sync.dma_start(wv_sb[:], wv[:])
        nc.sync.dma_start(bk_sb[:], bk[:])
        nc.sync.dma_start(bv_sb[:], bv[:])
        nc.vector.memset(ones_sb[:], 1.0)
        nc.vector.memset(zrhs_sb[:], 0.0)

        fct_ctx = ExitStack()
        fct_pool = fct_ctx.enter_context(
            tc.tile_pool(name="fct", bufs=1, space="PSUM"))
        fct = [fct_pool.tile([T, F], dt.float32, tag=f"fct{ec}", name=f"fct{ec}")
               for ec in range(EC)]
        # deterministically zero the accumulators (clears has_written too)
        for ec in range(EC):
            nc.tensor.matmul(fct[ec][:], ones_sb[0:1, :], zrhs_sb[0:1, :],
                             start=True, stop=False, skip_group_check=True)

        with tc.tile_pool(name="xp", bufs=4) as xp, \
             tc.tile_pool(name="cp", bufs=4) as cp, \
             tc.tile_pool(name="kvp", bufs=2, space="PSUM") as kvp, \
             tc.tile_pool(name="kbvb", bufs=2) as kbvb, \
             tc.tile_pool(name="depp", bufs=3) as depp, \
             tc.tile_pool(name="small", bufs=3) as small:

            for ti in range(NT):
                x_t = xp.tile([T, DC * T], dt.bfloat16, tag="x")
                nc.sync.dma_start(x_t[:], xr[ti])
                c_t = cp.tile([T, CST_W], dt.bfloat16, tag="c")
                nc.sync.dma_start(c_t[:], cstw[ti])

                k_ps = kvp.tile([T, F], dt.float32, tag="k")
                v_ps = kvp.tile([T, F], dt.float32, tag="v")
                for dc in range(DC):
                    nc.tensor.matmul(
                        k_ps[:], x_t[:, dc * T:(dc + 1) * T],
                        wk_sb[:, dc * F:(dc + 1) * F],
                        start=(dc == 0), stop=(dc == DC - 1))
                    nc.tensor.matmul(
                        v_ps[:], x_t[:, dc * T:(dc + 1) * T],
                        wv_sb[:, dc * F:(dc + 1) * F],
                        start=(dc == 0), stop=(dc == DC - 1))

                kb = kbvb.tile([T, F], dt.float32, tag="kb")
                nc.vector.tensor_add(kb[:], k_ps[:], bk_sb[:])
                vb = kbvb.tile([T, F], dt.float32, tag="vb")
                nc.vector.tensor_add(vb[:], v_ps[:], bv_sb[:])

                sq = kbvb.tile([T, F], dt.float32, tag="sq")
                nc.scalar.activation(sq[:], kb[:], Square)
                mag2 = small.tile([T, HG], dt.float32, tag="mag2")
                nc.vector.reduce_sum(
                    mag2[:], sq[:].rearrange("p (h e) -> p h e", h=HG),
                    axis=mybir.AxisListType.X)
                mag = small.tile([T, HG], dt.float32, tag="mag")
                nc.scalar.sqrt(mag[:], mag2[:])

                dep = depp.tile([T, F], dt.bfloat16, tag="dep")
                mag_b = bass.AP(mag.tensor, mag.offset,
                                [list(mag.ap[0]), [1, HG], [0, hd]])
                nc.vector.tensor_mul(
                    dep[:].rearrange("p (h e) -> p h e", h=HG),
                    vb[:].rearrange("p (h e) -> p h e", h=HG),
                    mag_b)

                last_tile = ti == NT - 1
                for ec in range(EC):
                    for si, (g0, c0, w) in enumerate(segs[ti]):
                        nc.tensor.matmul(
                            fct[ec][:, g0:g0 + w],
                            dep[:, ec * T:(ec + 1) * T],
                            c_t[:, c0:c0 + w],
                            start=False,
                            stop=last_tile and si == len(segs[ti]) - 1,
                            skip_group_check=True)

                if ti == 4:
                    # tail-phase constants on the ACT HWDGE ring, issued
                    # after the startup burst so they don't steal HBM BW
                    nc.scalar.dma_start(wo_sb[:], wo[:])
                    nc.scalar.dma_start(bo_sb[:], bo[:])
                    # g2_sb[p, bi*T + t] = g2tw[bi, p, t]
                    nc.scalar.dma_start(
                        g2_sb[:].rearrange("p (b t) -> p b t", b=NB),
                        g2tw.rearrange("b p t -> p b t"))

        # ---- tail: harvest field, project, gather, write ----
        for ec in range(EC):
            nc.scalar.copy(fct_sb[:, ec * F:(ec + 1) * F], fct[ec][:])

        with tc.tile_pool(name="pcp", bufs=2, space="PSUM") as pcp:
            for gc in range(4):
                for dcn in range(2):
                    p = pcp.tile([T, F], dt.float32, tag="p",
                                 name=f"pcp{gc}{dcn}")
                    for ec in range(EC):
                        nc.tensor.matmul(
                            p[:],
                            fct_sb[:, ec * F + gc * T:ec * F + (gc + 1) * T],
                            wo_sb[:, ec * D + dcn * F:ec * D + (dcn + 1) * F],
                            start=(ec == 0), stop=False)
                    nc.tensor.matmul(
                        p[:], ones_sb[0:1, :], bo_sb[0:1, dcn * F:(dcn + 1) * F],
                        start=False, stop=True)
                    nc.scalar.copy(pc_sb[gc][:, dcn * F:(dcn + 1) * F], p[:])
        fct_ctx.close()

        with tc.tile_pool(name="gat", bufs=6, space="PSUM") as gat, \
             tc.tile_pool(name="ob", bufs=4) as obp:
            for ti in range(NT):
                ob = obp.tile([T, D], dt.bfloat16, tag="ob")
                for dcn in range(2):
                    gt = gat.tile([T, F], dt.float32, tag="gt")
                    nblk = len(gblocks[ti])
                    for bi, (bgc, bidx) in enumerate(gblocks[ti]):
                        nc.tensor.matmul(
                            gt[:],
                            g2_sb[:, bidx * T:(bidx + 1) * T],
                            pc_sb[bgc][:, dcn * F:(dcn + 1) * F],
                            start=(bi == 0), stop=(bi == nblk - 1))
                    # split the PSUM->SBUF evictions across ACT and DVE
                    if dcn == 0:
                        nc.scalar.copy(ob[:, dcn * F:(dcn + 1) * F], gt[:])
                    else:
                        nc.vector.tensor_copy(ob[:, dcn * F:(dcn + 1) * F], gt[:])
                nc.sync.dma_start(out[ti * T:(ti + 1) * T, :], ob[:])

    nc.compile()
    return nc


_NC = None


def _compiled():
    global _NC
    if _NC is None:
        _NC = build_nc()
    return _NC


def kernel(x, Wqkv, bqkv, Wout, bout):
    x = np.asarray(x, dtype=np.float32)
    Wqkv = np.asarray(Wqkv, dtype=np.float32)
    bqkv = np.asarray(bqkv, dtype=np.float32)
    Wout = np.asarray(Wout, dtype=np.float32)
    bout = np.asarray(bout, dtype=np.float32)

    nc = _compiled()
    in_maps = _host_inputs(x, Wqkv, bqkv, Wout, bout)
    res = run_bass_kernel_spmd(nc, in_maps, core_ids=list(range(NCORES)))

    out = _combine(res)
    return out


def _combine(res):
    out = np.empty((B, N, D), np.float32)
    for b in range(B):
        out[b] = (res.results[2 * b]["out"].astype(np.float32)
                  + res.results[2 * b + 1]["out"].astype(np.float32))
    return out


def run_traced(x, Wqkv, bqkv, Wout, bout, **trace_kwargs):
    """Like kernel() but with NTFF tracing; returns (out, BassKernelResults)."""
    import ntff_shim  # noqa: F401  # registers the axon NTFF hook

    nc = _compiled()
    in_maps = _host_inputs(
        np.asarray(x, np.float32), np.asarray(Wqkv, np.float32),
        np.asarray(bqkv, np.float32), np.asarray(Wout, np.float32),
        np.asarray(bout, np.float32))
    res = run_bass_kernel_spmd(nc, in_maps, core_ids=list(range(NCORES)),
                               trace=True, **trace_kwargs)
    return _combine(res), res


# revision 16
# speedup vs baseline: 1.3687x; 1.0737x over previous
"""CausalFieldAttentionV2 on 8 TRN2 NeuronCores.

Math (per reference): qkv projection (q unused) -> per-head k magnitude ->
deposit = v * |k| -> scatter-add into a G=512-bin field by token position ->
circular causal conv along the field (FFT in the reference) -> gather back at
each token's bin -> output projection.

Key transformations used here:
- The FFT circular conv with a fixed kernel == multiplication by a circulant
  matrix with ~11 significant taps.  Scatter + conv fuse into one banded
  matrix CS[g, n] = k[(g - bin(n)) % G] applied to the deposits; per
  128-token tile only a <=64-wide g-window of CS is nonzero, so the whole
  scatter+conv is a windowed PE matmul against a host-precomputed compacted
  block.
- The output projection commutes with the gather (row replication), so it is
  applied on the tiny [G, D] field (16x fewer FLOPs than token space), then
  the gather is a 0/1 windowed PE matmul back to token space.

Sharding: 8 cores = batch (4) x head-group (2 x 8 heads).  Each core computes
a partial [N, D] output (its 8 heads' contribution); the host unshard step
sums the two partials per sample (bout is folded into the head-group-0 core
via a K=1 ones x bout matmul).

Compute dtype: bf16 matmul operands, fp32 PSUM accumulation and fp32
elementwise/bias math (measured end-to-end rel err ~5e-3 vs fp32 reference).
"""

from contextlib import ExitStack

import numpy as np
import ml_dtypes

import concourse.bass as bass
import concourse.mybir as mybir
import concourse.tile as tile
from concourse import bacc
from concourse.bass_utils import run_bass_kernel_spmd

B, N, D, H, hd, G = 4, 8192, 1024, 16, 64, 512
HG = 8            # heads per group
F = HG * hd       # 512 features per head-group
T = 128           # tokens per tile
NT = N // T       # 64 token tiles
DC = D // T       # 8 contraction chunks for the kv matmul
EC = F // T       # 4 feature chunks
SIGMA = 0.5
CST_W = 64        # compacted scatter+conv block width
NCORES = 8

bf16 = ml_dtypes.bfloat16
f32 = np.float32


# ---------------------------------------------------------------- host prep

def _field_index():
    seq = np.arange(N, dtype=np.float32)
    idx = (seq / np.float32(max(N - 1, 1)) * np.float32(G - 1)).astype(np.int32)
    return np.clip(idx, 0, G - 1)


def _causal_kernel():
    i = np.arange(G, dtype=np.float32)
    center = G // 2
    with np.errstate(over="ignore"):
        k = np.where(i <= center, np.exp(-(center - i) / np.float32(SIGMA)), 0.0)
    k = k.astype(np.float32)
    return k / (k.sum() + np.float32(1e-8))


def _scatter_plan():
    """Per token-tile: compacted CS block [T, CST_W] and PSUM column segments.

    CS[g, n] = k[(g - bin(n)) % G].  Returns (cstw [NT, T, CST_W] bf16,
    segs[ti] = [(g_start, col_start, width), ...]).
    """
    fidx = _field_index()
    k = _causal_kernel()
    taps = np.nonzero(k > 1e-12)[0]
    tmin, tmax = int(taps.min()), int(taps.max())
    cstw = np.zeros((NT, T, CST_W), np.float32)
    segs = []
    for ti in range(NT):
        b = fidx[ti * T:(ti + 1) * T]
        lo = int(b.min()) + tmin
        width = int(b.max()) + tmax - lo + 1
        assert width <= CST_W
        for j in range(T):
            for t in taps:
                cstw[ti, j, b[j] + t - lo] += k[t]
        lo_m = lo % G
        if lo_m + width <= G:
            segs.append([(lo_m, 0, width)])
        else:
            w1 = G - lo_m
            segs.append([(lo_m, 0, w1), (0, w1, width - w1)])
    return cstw.astype(bf16), segs


def _gather_plan():
    """Per token-tile: 128-aligned K=128 gather blocks (one per touched
    g-chunk, so lhsT and rhs share base partition 0).

    Returns (g2tw [NB, T, T] bf16, blocks[ti] = [(g_chunk, block_idx), ...]).
    """
    fidx = _field_index()
    g2_blocks = []
    blocks = []
    for ti in range(NT):
        b = fidx[ti * T:(ti + 1) * T]
        entry = []
        for gc in sorted(set(int(x) // T for x in np.unique(b))):
            blk = np.zeros((T, T), np.float32)
            for t in range(T):
                g = int(b[t])
                if g // T == gc:
                    blk[g - gc * T, t] = 1.0
            entry.append((gc, len(g2_blocks)))
            g2_blocks.append(blk)
        blocks.append(entry)
    g2tw = np.stack(g2_blocks).astype(bf16)
    return g2tw, blocks


_SCATTER = None
_GATHER = None


def _plans():
    global _SCATTER, _GATHER
    if _SCATTER is None:
        _SCATTER = _scatter_plan()
        _GATHER = _gather_plan()
    return _SCATTER, _GATHER


def _host_inputs(x, Wqkv, bqkv, Wout, bout):
    """Build the 8 per-core input maps."""
    (cstw, _), (g2tw, _) = _plans()
    cstw_flat = np.ascontiguousarray(cstw)             # [NT, T, CST_W] bf16
    g2tw_flat = np.ascontiguousarray(g2tw)             # [NB, 64, T] bf16

    xr = []
    for b in range(B):
        # xr[ti, p, dc*T + t] = x[b, ti*T + t, dc*T + p]
        a = np.ascontiguousarray(x[b].T)               # [D, N]
        a = a.reshape(DC, T, NT, T).transpose(2, 1, 0, 3).reshape(NT, T, DC * T)
        xr.append(np.ascontiguousarray(a.astype(bf16)))

    per_hg = []
    for hg in range(2):
        rk = slice(D + hg * F, D + (hg + 1) * F)
        rv = slice(2 * D + hg * F, 2 * D + (hg + 1) * F)
        # wk[p, dc*F + f] = Wqkv[D + hg*F + f, dc*T + p]
        wk = np.ascontiguousarray(
            Wqkv[rk].T.reshape(DC, T, F).transpose(1, 0, 2).reshape(T, DC * F)
        ).astype(bf16)
        wv = np.ascontiguousarray(
            Wqkv[rv].T.reshape(DC, T, F).transpose(1, 0, 2).reshape(T, DC * F)
        ).astype(bf16)
        # wo[p, ec*D + d] = Wout[d, hg*F + ec*T + p]
        wo = np.ascontiguousarray(
            Wout[:, hg * F:(hg + 1) * F].T.reshape(EC, T, D)
            .transpose(1, 0, 2).reshape(T, EC * D)
        ).astype(bf16)
        bk = np.ascontiguousarray(
            np.broadcast_to(bqkv[rk][None, :], (T, F))
        ).astype(f32)
        bv = np.ascontiguousarray(
            np.broadcast_to(bqkv[rv][None, :], (T, F))
        ).astype(f32)
        bo = (bout if hg == 0 else np.zeros_like(bout)).reshape(1, D).astype(bf16)
        per_hg.append((wk, wv, wo, bk, bv, np.ascontiguousarray(bo)))

    in_maps = []
    for core in range(NCORES):
        b, hg = divmod(core, 2)
        wk, wv, wo, bk, bv, bo = per_hg[hg]
        in_maps.append({
            "xr": xr[b], "cstw": cstw_flat, "g2tw": g2tw_flat,
            "wk": wk, "wv": wv, "wo": wo, "bk": bk, "bv": bv, "bo": bo,
        })
    return in_maps


# ---------------------------------------------------------------- device

def build_nc():
    (_, segs), (g2tw_np, gblocks) = _plans()
    NB = g2tw_np.shape[0]
    dt = mybir.dt

    # Back-half burst schedule.  The conv kernel's +G/2 center shift means
    # fct g-chunk gc's columns stop receiving contributions at main-loop
    # tile ~{2: 18, 3: 34, 0: 50, 1: 64}[gc]; emit each chunk's output
    # projection + gathers as soon as its columns are final so the output
    # DMA overlaps the remaining main loop.  A gather tile goes in the burst
    # of its latest-ready g-chunk.
    GC_RANK = {2: 0, 3: 1, 0: 2, 1: 3}
    RANK_GC = {r: gc for gc, r in GC_RANK.items()}
    BURST_AFTER = {0: 20, 1: 36, 2: 52}          # rank -> main-loop tile
    burst_tiles = {r: [] for r in range(4)}
    for ti in range(NT):
        burst_tiles[max(GC_RANK[gc] for gc, _ in gblocks[ti])].append(ti)

    nc = bacc.Bacc("TRN2", target_bir_lowering=False, debug=False,
                   num_devices=NCORES)

    xr = nc.dram_tensor("xr", [NT, T, DC * T], dt.bfloat16,
                        kind="ExternalInput").ap()
    cstw = nc.dram_tensor("cstw", [NT, T, CST_W], dt.bfloat16,
                          kind="ExternalInput").ap()
    g2tw = nc.dram_tensor("g2tw", [NB, T, T], dt.bfloat16,
                          kind="ExternalInput").ap()
    wk = nc.dram_tensor("wk", [T, DC * F], dt.bfloat16, kind="ExternalInput").ap()
    wv = nc.dram_tensor("wv", [T, DC * F], dt.bfloat16, kind="ExternalInput").ap()
    wo = nc.dram_tensor("wo", [T, EC * D], dt.bfloat16, kind="ExternalInput").ap()
    bk = nc.dram_tensor("bk", [T, F], dt.float32, kind="ExternalInput").ap()
    bv = nc.dram_tensor("bv", [T, F], dt.float32, kind="ExternalInput").ap()
    bo = nc.dram_tensor("bo", [1, D], dt.bfloat16, kind="ExternalInput").ap()
    out = nc.dram_tensor("out", [N, D], dt.bfloat16, kind="ExternalOutput").ap()

    Square = mybir.ActivationFunctionType.Square

    with tile.TileContext(nc) as tc, ExitStack() as ctx:
        const = ctx.enter_context(tc.tile_pool(name="const", bufs=1))

        wk_sb = const.tile([T, DC * F], dt.bfloat16, tag="wk")
        wv_sb = const.tile([T, DC * F], dt.bfloat16, tag="wv")
        wo_sb = const.tile([T, EC * D], dt.bfloat16, tag="wo")
        bk_sb = const.tile([T, F], dt.float32, tag="bk")
        bv_sb = const.tile([T, F], dt.float32, tag="bv")
        bo_sb = const.tile([1, D], dt.bfloat16, tag="bo")
        g2_sb = const.tile([T, NB * T], dt.bfloat16, tag="g2")
        ones_sb = const.tile([1, T], dt.bfloat16, tag="ones")
        zrhs_sb = const.tile([1, F], dt.bfloat16, tag="zrhs")
        fct_sb = const.tile([T, EC * F], dt.bfloat16, tag="fct_sb")
        pc_sb = [const.tile([T, D], dt.bfloat16, tag=f"pc{gc}", name=f"pc_sb{gc}")
                 for gc in range(4)]

        nc.vector.memset(ones_sb[:], 1.0)
        nc.vector.memset(zrhs_sb[:], 0.0)

        fct_ctx = ExitStack()
        fct_pool = fct_ctx.enter_context(
            tc.tile_pool(name="fct", bufs=1, space="PSUM"))
        fct = [fct_pool.tile([T, F], dt.float32, tag=f"fct{ec}", name=f"fct{ec}")
               for ec in range(EC)]
        # deterministically zero the accumulators (clears has_written too)
        for ec in range(EC):
            nc.tensor.matmul(fct[ec][:], ones_sb[0:1, :], zrhs_sb[0:1, :],
                             start=True, stop=False, skip_group_check=True)

        with tc.tile_pool(name="xp", bufs=4) as xp, \
             tc.tile_pool(name="cp", bufs=4) as cp, \
             tc.tile_pool(name="kvp", bufs=2, space="PSUM") as kvp, \
             tc.tile_pool(name="kbvb", bufs=2) as kbvb, \
             tc.tile_pool(name="depp", bufs=3) as depp, \
             tc.tile_pool(name="small", bufs=3) as small:

            # All constants stream on the ACT HWDGE ring (wk first — the
            # first k-matmul needs it); the x/cs tile stream has the SP
            # ring to itself, so x0 lands within ~1.5us.
            nc.scalar.dma_start(wk_sb[:], wk[:])
            nc.scalar.dma_start(wv_sb[:], wv[:])
            nc.scalar.dma_start(bk_sb[:], bk[:])
            nc.scalar.dma_start(bv_sb[:], bv[:])
            nc.scalar.dma_start(wo_sb[:], wo[:])
            nc.scalar.dma_start(bo_sb[:], bo[:])
            # g2_sb[p, bi*T + t] = g2tw[bi, p, t]
            nc.scalar.dma_start(
                g2_sb[:].rearrange("p (b t) -> p b t", b=NB),
                g2tw.rearrange("b p t -> p b t"))

            for ti in range(NT):
                x_t = xp.tile([T, DC * T], dt.bfloat16, tag="x")
                nc.sync.dma_start(x_t[:], xr[ti])
                c_t = cp.tile([T, CST_W], dt.bfloat16, tag="c")
                nc.sync.dma_start(c_t[:], cstw[ti])

                k_ps = kvp.tile([T, F], dt.float32, tag="k")
                v_ps = kvp.tile([T, F], dt.float32, tag="v")
                for dc in range(DC):
                    nc.tensor.matmul(
                        k_ps[:], x_t[:, dc * T:(dc + 1) * T],
                        wk_sb[:, dc * F:(dc + 1) * F],
                        start=(dc == 0), stop=(dc == DC - 1))
                for dc in range(DC):
                    nc.tensor.matmul(
                        v_ps[:], x_t[:, dc * T:(dc + 1) * T],
                        wv_sb[:, dc * F:(dc + 1) * F],
                        start=(dc == 0), stop=(dc == DC - 1))

                kb = kbvb.tile([T, F], dt.float32, tag="kb")
                nc.vector.tensor_add(kb[:], k_ps[:], bk_sb[:])
                vb = kbvb.tile([T, F], dt.float32, tag="vb")
                nc.vector.tensor_add(vb[:], v_ps[:], bv_sb[:])

                sq = kbvb.tile([T, F], dt.float32, tag="sq")
                nc.scalar.activation(sq[:], kb[:], Square)
                mag2 = small.tile([T, HG], dt.float32, tag="mag2")
                nc.vector.reduce_sum(
                    mag2[:], sq[:].rearrange("p (h e) -> p h e", h=HG),
                    axis=mybir.AxisListType.X)
                mag = small.tile([T, HG], dt.float32, tag="mag")
                nc.scalar.sqrt(mag[:], mag2[:])

                dep = depp.tile([T, F], dt.bfloat16, tag="dep")
                mag_b = bass.AP(mag.tensor, mag.offset,
                                [list(mag.ap[0]), [1, HG], [0, hd]])
                nc.vector.tensor_mul(
                    dep[:].rearrange("p (h e) -> p h e", h=HG),
                    vb[:].rearrange("p (h e) -> p h e", h=HG),
                    mag_b)

                # harvest fct columns as soon as they are final (the conv
                # shift means gc2@18, gc3@34, gc0@50); only ACT copies, no
                # PSUM pressure — shortens the tail ramp
                for hgc, hafter in ((2, 20), (3, 36), (0, 52)):
                    if ti == hafter:
                        for ec in range(EC):
                            nc.scalar.copy(
                                fct_sb[:, ec * F + hgc * T:ec * F + (hgc + 1) * T],
                                fct[ec][:, hgc * T:(hgc + 1) * T])

                last_tile = ti == NT - 1
                for ec in range(EC):
                    for si, (g0, c0, w) in enumerate(segs[ti]):
                        nc.tensor.matmul(
                            fct[ec][:, g0:g0 + w],
                            dep[:, ec * T:(ec + 1) * T],
                            c_t[:, c0:c0 + w],
                            start=False,
                            stop=last_tile and si == len(segs[ti]) - 1,
                            skip_group_check=True)

        # ---- tail: harvest the last field chunk, project, gather, write ----
        for ec in range(EC):
            nc.scalar.copy(fct_sb[:, ec * F + T:ec * F + 2 * T],
                           fct[ec][:, T:2 * T])

        with tc.tile_pool(name="pcp", bufs=4, space="PSUM") as pcp:
            for gc in range(4):
                for dcn in range(2):
                    p = pcp.tile([T, F], dt.float32, tag="p",
                                 name=f"pcp{gc}{dcn}")
                    for ec in range(EC):
                        nc.tensor.matmul(
                            p[:],
                            fct_sb[:, ec * F + gc * T:ec * F + (gc + 1) * T],
                            wo_sb[:, ec * D + dcn * F:ec * D + (dcn + 1) * F],
                            start=(ec == 0), stop=False)
                    nc.tensor.matmul(
                        p[:], ones_sb[0:1, :], bo_sb[0:1, dcn * F:(dcn + 1) * F],
                        start=False, stop=True)
                    nc.scalar.copy(pc_sb[gc][:, dcn * F:(dcn + 1) * F], p[:])
        fct_ctx.close()

        with tc.tile_pool(name="gat", bufs=8, space="PSUM") as gat, \
             tc.tile_pool(name="ob", bufs=6) as obp:
            for ti in range(NT):
                ob = obp.tile([T, D], dt.bfloat16, tag="ob")
                for dcn in range(2):
                    gt = gat.tile([T, F], dt.float32, tag="gt")
                    nblk = len(gblocks[ti])
                    for bi, (bgc, bidx) in enumerate(gblocks[ti]):
                        nc.tensor.matmul(
                            gt[:],
                            g2_sb[:, bidx * T:(bidx + 1) * T],
                            pc_sb[bgc][:, dcn * F:(dcn + 1) * F],
                            start=(bi == 0), stop=(bi == nblk - 1))
                    # split the PSUM->SBUF evictions across ACT and DVE
                    if dcn == 0:
                        nc.scalar.copy(ob[:, dcn * F:(dcn + 1) * F], gt[:])
                    else:
                        nc.vector.tensor_copy(ob[:, dcn * F:(dcn + 1) * F], gt[:])
                nc.sync.dma_start(out[ti * T:(ti + 1) * T, :], ob[:])

    nc.compile()
    return nc


_NC = None


def _compiled():
    global _NC
    if _NC is None:
        _NC = build_nc()
    return _NC


def kernel(x, Wqkv, bqkv, Wout, bout):
    x = np.asarray(x, dtype=np.float32)
    Wqkv = np.asarray(Wqkv, dtype=np.float32)
    bqkv = np.asarray(bqkv, dtype=np.float32)
    Wout = np.asarray(Wout, dtype=np.float32)
    bout = np.asarray(bout, dtype=np.float32)

    nc = _compiled()
    in_maps = _host_inputs(x, Wqkv, bqkv, Wout, bout)
    res = run_bass_kernel_spmd(nc, in_maps, core_ids=list(range(NCORES)))

    out = _combine(res)
    return out


def _combine(res):
    out = np.empty((B, N, D), np.float32)
    for b in range(B):
        out[b] = (res.results[2 * b]["out"].astype(np.float32)
                  + res.results[2 * b + 1]["out"].astype(np.float32))
    return out


def run_traced(x, Wqkv, bqkv, Wout, bout, **trace_kwargs):
    """Like kernel() but with NTFF tracing; returns (out, BassKernelResults)."""
    import ntff_shim  # noqa: F401  # registers the axon NTFF hook

    nc = _compiled()
    in_maps = _host_inputs(
        np.asarray(x, np.float32), np.asarray(Wqkv, np.float32),
        np.asarray(bqkv, np.float32), np.asarray(Wout, np.float32),
        np.asarray(bout, np.float32))
    res = run_bass_kernel_spmd(nc, in_maps, core_ids=list(range(NCORES)),
                               trace=True, **trace_kwargs)
    return _combine(res), res
